# revision 5
# baseline (speedup 1.0000x reference)
"""Butterfly (nn_Butterfly) forward as a single dense matmul on 8 TRN2 cores.

The reference butterfly network is linear in x: forward(x) == x @ M + b where
M = forward(I_1024) with b=0.  M is built on the host from the ~16KB params.

v4 device kernel (from v3 trace analysis: 12us DMA-limited ramp with a cold
PE, 8.5us end-of-kernel semaphore teardown, f32 stores):
  - bf16 stores (4.2MB instead of 8.4MB per core), host upcasts to f32.
  - 8 junk warm-up matmuls issued at t=0 into a scratch psum bank so the
    HAM clock gate reaches 8/8 (2.4 GHz) before the first real matmul.
  - loads split across two HWDGE rings: M (per-kt chunks) + bulk x on the
    sync ring, first x tiles on the vector ring, bias on gpsimd; stores on
    the scalar ring.  First real matmul gates only on m[kt0] + x[t0:2].
  - explicit tile reuse (3 psum set pairs, 2 output sbuf buffers) instead
    of per-btile pool.tile() calls -- the end-of-kernel teardown barrier
    scales with the number of tile objects (~52 in v3).
  - kt-outer/jc-inner steady state: one x LDWEIGHTS serves both jc matmuls.
"""

import numpy as np

N = 1024
B_FULL = 16384
N_CORES = 8
B_CORE = B_FULL // N_CORES  # 2048
N_BTILES = B_CORE // 128  # 16
N_KT = 8  # k-tiles (K=128 each)
N_WARMUP_MM = 8


# ---------------------------------------------------------------------------
# Host side: collapse the butterfly network to a single matrix
# ---------------------------------------------------------------------------

def _abcd_offsets(n):
    offs = []
    off = 0
    m = n
    while m >= 2:
        offs.append((m, off))
        off += 2 * m
        m //= 2
    return offs, off


def _np_forward(x, perm_logit, abcd, b):
    """Float64 numpy port of reference._forward (op-for-op)."""
    x = np.asarray(x, np.float64)
    perm_logit = np.asarray(perm_logit, np.float64)
    abcd = np.asarray(abcd, np.float64)
    b = np.asarray(b, np.float64)
    n = x.shape[-1]
    Bn = x.shape[0]
    offs, _ = _abcd_offsets(n)
    h = np.stack([x, np.zeros_like(x)], axis=-1)
    perm_sizes = [m for (m, _) in offs if m >= 4]
    for d in range(perm_logit.shape[0]):
        p = 1.0 / (1.0 + np.exp(-perm_logit[d]))
        for m in reversed(perm_sizes):
            h = h.reshape(Bn, n // m, m, 2)
            eo = np.concatenate([h[:, :, 0::2], h[:, :, 1::2]], axis=2)
            h = (1 - p[0]) * h + p[0] * eo
            h1, h2 = h[:, :, : m // 2], h[:, :, m // 2 :]
            h1 = (1 - p[1]) * h1 + p[1] * h1[:, :, ::-1]
            h2 = (1 - p[2]) * h2 + p[2] * h2[:, :, ::-1]
            h = np.concatenate([h1, h2], axis=2).reshape(Bn, n, 2)
        for (m, off) in reversed(offs):
            ABCD = abcd[d, off : off + 2 * m].reshape(2, 2, m // 2, 2)
            hv = h.reshape(Bn, n // m, 2, m // 2, 2)
            xr, xi = hv[..., 0], hv[..., 1]
            Ar, Ai = ABCD[..., 0], ABCD[..., 1]
            yr = np.einsum("ijk,bnjk->bnik", Ar, xr) - np.einsum(
                "ijk,bnjk->bnik", Ai, xi
            )
            yi = np.einsum("ijk,bnjk->bnik", Ar, xi) + np.einsum(
                "ijk,bnjk->bnik", Ai, xr
            )
            h = np.stack([yr, yi], axis=-1).reshape(Bn, n, 2)
    return b + h[..., 0]


def _build_matrix(perm_logit, abcd):
    """M (f32, [k, j]) with forward(x) == x @ M + b."""
    I = np.eye(N, dtype=np.float64)
    M = _np_forward(I, perm_logit, abcd, np.zeros((N,), np.float64))
    return M.astype(np.float32)


# ---------------------------------------------------------------------------
# Device kernel
# ---------------------------------------------------------------------------

_BUILT = {}


def _build_nc():
    import concourse.bacc as bacc
    import concourse.mybir as mybir
    from concourse.tile import TileContext

    f32 = mybir.dt.float32
    bf16 = mybir.dt.bfloat16

    nc = bacc.Bacc(None, target_bir_lowering=False)

    xb_d = nc.dram_tensor("xb", [128, N_BTILES, N_KT, 128], bf16, kind="ExternalInput")
    m_d = nc.dram_tensor("m", [128, N_KT, N], bf16, kind="ExternalInput")
    b_d = nc.dram_tensor("bias", [128, N], f32, kind="ExternalInput")
    o_d = nc.dram_tensor("out", [B_CORE, N], bf16, kind="ExternalOutput")

    with TileContext(nc) as tc:
        with (
            tc.tile_pool(name="const", bufs=1) as const,
            tc.tile_pool(name="ps", bufs=1, space="PSUM") as ppool,
        ):
            m_sb = const.tile([128, N_KT, N], bf16)
            xb_sb = const.tile([128, N_BTILES, N_KT, 128], bf16)
            bias_sb = const.tile([128, N], f32)
            junk_sb = const.tile([128, 512], bf16)
            out_sb = [
                const.tile([128, N], bf16, name=f"osb{i}", tag=f"osb{i}")
                for i in range(2)
            ]

            scratch_ps = ppool.tile([128, 512], f32, name="scratch", tag="scratch")
            po = [
                [
                    ppool.tile([128, 512], f32, name=f"po{s}_{jc}", tag=f"po{s}_{jc}")
                    for jc in range(2)
                ]
                for s in range(3)
            ]

            # PE warm-up: 8 junk matmuls (~3.4us at the cold 1.2 GHz clock)
            # flip the HAM clock gate to 8/8 while the first DMAs land.
            nc.gpsimd.memset(junk_sb[:], 0.0)
            for _ in range(N_WARMUP_MM):
                nc.tensor.matmul(
                    scratch_ps[:], junk_sb[:, 0:128], junk_sb[:], start=True, stop=True
                )

            # Loads in consumption order on the sync HWDGE ring: m[kt0],
            # the two ramp x btiles, then the remaining M chunks.  Bulk x
            # and bias go on the scalar HWDGE ring (in front of the
            # stores), so both rings stream concurrently from t=0.
            nc.sync.dma_start(m_sb[:, 0], m_d[:, 0])
            nc.sync.dma_start(xb_sb[:, 0:1], xb_d[:, 0:1])
            nc.sync.dma_start(xb_sb[:, 1:2], xb_d[:, 1:2])
            for kt in range(1, N_KT):
                nc.sync.dma_start(m_sb[:, kt], m_d[:, kt])
            nc.scalar.dma_start(bias_sb[:], b_d[:])
            nc.scalar.dma_start(xb_sb[:, 2:9], xb_d[:, 2:9])
            nc.scalar.dma_start(xb_sb[:, 9:16], xb_d[:, 9:16])

            def mm(p, t, kt, jc):
                js = slice(jc * 512, (jc + 1) * 512)
                nc.tensor.matmul(
                    p[jc][:],
                    xb_sb[:, t, kt, :],
                    m_sb[:, kt, js],
                    start=(kt == 0),
                    stop=(kt == N_KT - 1),
                )

            def evict_jc(t, p, jc):
                # per-jc add + store: each half leaves as soon as its psum
                # group closes, shortening the end-of-kernel tail.
                osb = out_sb[t % 2]
                js = slice(jc * 512, (jc + 1) * 512)
                nc.vector.tensor_add(osb[:, js], p[jc][:], bias_sb[:, js])
                nc.scalar.dma_start(o_d[t * 128 : (t + 1) * 128, js], osb[:, js])

            def evict(t, p):
                for jc in range(2):
                    evict_jc(t, p, jc)

            # Ramp: btiles 0-1 kt-outer, consuming each M chunk as it lands.
            for kt in range(N_KT):
                for t in range(2):
                    for jc in range(2):
                        mm(po[t], t, kt, jc)
            for t in range(2):
                evict(t, po[t])

            # Steady state: kt-outer / jc-inner (one LDWEIGHTS per x tile),
            # psum sets round-robin t%3, output sbuf ping-pong t%2.
            for t in range(2, N_BTILES - 1):
                p = po[t % 3]
                for kt in range(N_KT):
                    for jc in range(2):
                        mm(p, t, kt, jc)
                evict(t, p)

            # Last btile jc-outer: jc0 closes 8 matmuls early, so its
            # evict + store overlap jc1's matmuls.
            t = N_BTILES - 1
            p = po[t % 3]
            for jc in range(2):
                for kt in range(N_KT):
                    mm(p, t, kt, jc)
                evict_jc(t, p, jc)

    nc.compile()
    return nc


def _get_nc():
    if "v4" not in _BUILT:
        _BUILT["v4"] = _build_nc()
    return _BUILT["v4"]


LAST_RUN = {}


def _install_axon_ntff_shim():
    """Provide the missing ``antenv.axon_hooks`` module so
    ``run_bass_kernel_spmd(trace=True)`` can capture NTFF profiles under
    axon.  The hook drives ``axon_{start,stop}_nrt_profile`` in
    libaxon_pjrt.so directly (same ABI trn_boot uses)."""
    import contextlib
    import ctypes
    import sys
    import types

    if "antenv.axon_hooks" in sys.modules:
        return
    so_path = "/opt/axon/libaxon_pjrt.so"
    lib = ctypes.CDLL(so_path)
    if not hasattr(lib, "axon_start_nrt_profile"):
        raise RuntimeError("libaxon_pjrt.so lacks axon_start_nrt_profile")
    lib.axon_start_nrt_profile.argtypes = [
        ctypes.POINTER(ctypes.c_int64),
        ctypes.c_size_t,
    ]
    lib.axon_start_nrt_profile.restype = ctypes.c_int64
    lib.axon_stop_nrt_profile.argtypes = [ctypes.c_char_p]
    lib.axon_stop_nrt_profile.restype = ctypes.c_int64

    @contextlib.contextmanager
    def _hook(output_dir, device_ids):
        import jax

        jax.devices()
        if device_ids:
            ids = (ctypes.c_int64 * len(device_ids))(*device_ids)
            rc = lib.axon_start_nrt_profile(ids, len(device_ids))
        else:
            rc = lib.axon_start_nrt_profile(None, 0)
        if rc != 0:
            raise RuntimeError(f"axon_start_nrt_profile rc={rc}")
        try:
            yield
        finally:
            n = lib.axon_stop_nrt_profile(str(output_dir).encode())
            print(f"ntff profile: {n} file(s) written to {output_dir}")

    mod = types.ModuleType("antenv.axon_hooks")
    mod.get_axon_ntff_profile_hook = lambda: _hook
    mod.set_axon_ntff_profile_hook = lambda h: None
    sys.modules["antenv.axon_hooks"] = mod
    import antenv

    antenv.axon_hooks = mod


def kernel(x, perm_logit, abcd, b, _trace=False):
    import ml_dtypes
    import concourse.bass_utils as bass_utils
    from concourse.bass_utils import run_bass_kernel_spmd

    if _trace:
        try:
            _install_axon_ntff_shim()
            bass_utils.upload_artifacts = lambda tmpdir: tmpdir
        except Exception as e:  # degrade to untraced run
            print("trace setup failed:", e)
            _trace = False

    x = np.ascontiguousarray(np.asarray(x, np.float32))
    M = _build_matrix(perm_logit, abcd)  # [k, j] f32

    # [k, j] -> [p, kt, j] with k = kt*128 + p
    m_in = np.ascontiguousarray(
        M.reshape(N_KT, 128, N).transpose(1, 0, 2).astype(ml_dtypes.bfloat16)
    )

    xb = x.astype(ml_dtypes.bfloat16)  # [B_FULL, N]
    # per-core shard -> [p, t, kt, b] with row = t*128+b, col = kt*128+p
    def x_layout(a, c):
        s = a[c * B_CORE : (c + 1) * B_CORE]
        return np.ascontiguousarray(
            s.reshape(N_BTILES, 128, N_KT, 128).transpose(3, 0, 2, 1)
        )

    bias_in = np.ascontiguousarray(
        np.broadcast_to(np.asarray(b, np.float32), (128, N))
    )

    nc = _get_nc()
    in_maps = [
        {
            "xb": x_layout(xb, c),
            "m": m_in,
            "bias": bias_in,
        }
        for c in range(N_CORES)
    ]
    res = run_bass_kernel_spmd(
        nc, in_maps, core_ids=list(range(N_CORES)), trace=_trace
    )
    LAST_RUN["results"] = res
    LAST_RUN["exec_time_ns"] = res.exec_time_ns
    out = np.concatenate(
        [np.asarray(r["out"]).astype(np.float32) for r in res.results], axis=0
    )
    return out


# revision 6
# speedup vs baseline: 1.1199x; 1.1199x over previous
"""Butterfly (nn_Butterfly) forward as a single dense matmul on 8 TRN2 cores.

The reference butterfly network is linear in x: forward(x) == x @ M + b where
M = forward(I_1024) with b=0.  M is built on the host from the ~16KB params.

v4 device kernel (from v3 trace analysis: 12us DMA-limited ramp with a cold
PE, 8.5us end-of-kernel semaphore teardown, f32 stores):
  - bf16 stores (4.2MB instead of 8.4MB per core), host upcasts to f32.
  - 8 junk warm-up matmuls issued at t=0 into a scratch psum bank so the
    HAM clock gate reaches 8/8 (2.4 GHz) before the first real matmul.
  - loads split across two HWDGE rings: M (per-kt chunks) + bulk x on the
    sync ring, first x tiles on the vector ring, bias on gpsimd; stores on
    the scalar ring.  First real matmul gates only on m[kt0] + x[t0:2].
  - explicit tile reuse (3 psum set pairs, 2 output sbuf buffers) instead
    of per-btile pool.tile() calls -- the end-of-kernel teardown barrier
    scales with the number of tile objects (~52 in v3).
  - kt-outer/jc-inner steady state: one x LDWEIGHTS serves both jc matmuls.
"""

import numpy as np

N = 1024
B_FULL = 16384
N_CORES = 8
B_CORE = B_FULL // N_CORES  # 2048
N_BTILES = B_CORE // 128  # 16
N_KT = 8  # k-tiles (K=128 each)
N_WARMUP_MM = 8


# ---------------------------------------------------------------------------
# Host side: collapse the butterfly network to a single matrix
# ---------------------------------------------------------------------------

def _abcd_offsets(n):
    offs = []
    off = 0
    m = n
    while m >= 2:
        offs.append((m, off))
        off += 2 * m
        m //= 2
    return offs, off


def _np_forward(x, perm_logit, abcd, b):
    """Float64 numpy port of reference._forward (op-for-op)."""
    x = np.asarray(x, np.float64)
    perm_logit = np.asarray(perm_logit, np.float64)
    abcd = np.asarray(abcd, np.float64)
    b = np.asarray(b, np.float64)
    n = x.shape[-1]
    Bn = x.shape[0]
    offs, _ = _abcd_offsets(n)
    h = np.stack([x, np.zeros_like(x)], axis=-1)
    perm_sizes = [m for (m, _) in offs if m >= 4]
    for d in range(perm_logit.shape[0]):
        p = 1.0 / (1.0 + np.exp(-perm_logit[d]))
        for m in reversed(perm_sizes):
            h = h.reshape(Bn, n // m, m, 2)
            eo = np.concatenate([h[:, :, 0::2], h[:, :, 1::2]], axis=2)
            h = (1 - p[0]) * h + p[0] * eo
            h1, h2 = h[:, :, : m // 2], h[:, :, m // 2 :]
            h1 = (1 - p[1]) * h1 + p[1] * h1[:, :, ::-1]
            h2 = (1 - p[2]) * h2 + p[2] * h2[:, :, ::-1]
            h = np.concatenate([h1, h2], axis=2).reshape(Bn, n, 2)
        for (m, off) in reversed(offs):
            ABCD = abcd[d, off : off + 2 * m].reshape(2, 2, m // 2, 2)
            hv = h.reshape(Bn, n // m, 2, m // 2, 2)
            xr, xi = hv[..., 0], hv[..., 1]
            Ar, Ai = ABCD[..., 0], ABCD[..., 1]
            yr = np.einsum("ijk,bnjk->bnik", Ar, xr) - np.einsum(
                "ijk,bnjk->bnik", Ai, xi
            )
            yi = np.einsum("ijk,bnjk->bnik", Ar, xi) + np.einsum(
                "ijk,bnjk->bnik", Ai, xr
            )
            h = np.stack([yr, yi], axis=-1).reshape(Bn, n, 2)
    return b + h[..., 0]


def _build_matrix(perm_logit, abcd):
    """M (f32, [k, j]) with forward(x) == x @ M + b."""
    I = np.eye(N, dtype=np.float64)
    M = _np_forward(I, perm_logit, abcd, np.zeros((N,), np.float64))
    return M.astype(np.float32)


# ---------------------------------------------------------------------------
# Device kernel
# ---------------------------------------------------------------------------

_BUILT = {}


def _build_nc():
    import concourse.bacc as bacc
    import concourse.mybir as mybir
    from concourse.tile import TileContext

    f32 = mybir.dt.float32
    bf16 = mybir.dt.bfloat16

    nc = bacc.Bacc(None, target_bir_lowering=False)

    xb_d = nc.dram_tensor("xb", [128, N_BTILES, N_KT, 128], bf16, kind="ExternalInput")
    m_d = nc.dram_tensor("m", [128, N_KT, N], bf16, kind="ExternalInput")
    b_d = nc.dram_tensor("bias", [128, N], f32, kind="ExternalInput")
    o_d = nc.dram_tensor("out", [B_CORE, N], bf16, kind="ExternalOutput")

    with TileContext(nc) as tc:
        with (
            tc.tile_pool(name="const", bufs=1) as const,
            tc.tile_pool(name="ps", bufs=1, space="PSUM") as ppool,
        ):
            m_sb = const.tile([128, N_KT, N], bf16)
            xb_sb = const.tile([128, N_BTILES, N_KT, 128], bf16)
            bias_sb = const.tile([128, N], f32)
            junk_sb = const.tile([128, 512], bf16)
            out_sb = [
                const.tile([128, N], bf16, name=f"osb{i}", tag=f"osb{i}")
                for i in range(2)
            ]

            scratch_ps = ppool.tile([128, 512], f32, name="scratch", tag="scratch")
            po = [
                [
                    ppool.tile([128, 512], f32, name=f"po{s}_{jc}", tag=f"po{s}_{jc}")
                    for jc in range(2)
                ]
                for s in range(3)
            ]

            # PE warm-up: 8 junk matmuls (~3.4us at the cold 1.2 GHz clock)
            # flip the HAM clock gate to 8/8 while the first DMAs land.
            nc.gpsimd.memset(junk_sb[:], 0.0)
            for _ in range(N_WARMUP_MM):
                nc.tensor.matmul(
                    scratch_ps[:], junk_sb[:, 0:128], junk_sb[:], start=True, stop=True
                )

            # Loads in consumption order on the sync HWDGE ring: m[kt0],
            # the two ramp x btiles, then the remaining M chunks.  Bulk x
            # and bias go on the scalar HWDGE ring (in front of the
            # stores), so both rings stream concurrently from t=0.
            nc.sync.dma_start(m_sb[:, 0], m_d[:, 0])
            nc.sync.dma_start(xb_sb[:, 0:1], xb_d[:, 0:1])
            nc.sync.dma_start(xb_sb[:, 1:2], xb_d[:, 1:2])
            for kt in range(1, N_KT):
                nc.sync.dma_start(m_sb[:, kt], m_d[:, kt])
            nc.sync.dma_start(xb_sb[:, 2:9], xb_d[:, 2:9])
            nc.sync.dma_start(xb_sb[:, 9:16], xb_d[:, 9:16])
            nc.scalar.dma_start(bias_sb[:], b_d[:])

            def mm(p, t, kt, jc):
                js = slice(jc * 512, (jc + 1) * 512)
                nc.tensor.matmul(
                    p[jc][:],
                    xb_sb[:, t, kt, :],
                    m_sb[:, kt, js],
                    start=(kt == 0),
                    stop=(kt == N_KT - 1),
                )

            def evict_jc(t, p, jc):
                # per-jc add + store: each half leaves as soon as its psum
                # group closes, shortening the end-of-kernel tail.
                osb = out_sb[t % 2]
                js = slice(jc * 512, (jc + 1) * 512)
                nc.vector.tensor_add(osb[:, js], p[jc][:], bias_sb[:, js])
                nc.scalar.dma_start(o_d[t * 128 : (t + 1) * 128, js], osb[:, js])

            def evict(t, p):
                for jc in range(2):
                    evict_jc(t, p, jc)

            # Ramp: btiles 0-1 kt-outer, consuming each M chunk as it lands.
            for kt in range(N_KT):
                for t in range(2):
                    for jc in range(2):
                        mm(po[t], t, kt, jc)
            for t in range(2):
                evict(t, po[t])

            # Steady state: kt-outer / jc-inner (one LDWEIGHTS per x tile),
            # psum sets round-robin t%3, output sbuf ping-pong t%2.
            for t in range(2, N_BTILES - 1):
                p = po[t % 3]
                for kt in range(N_KT):
                    for jc in range(2):
                        mm(p, t, kt, jc)
                evict(t, p)

            # Last btile jc-outer: jc0 closes 8 matmuls early, so its
            # evict + store overlap jc1's matmuls.
            t = N_BTILES - 1
            p = po[t % 3]
            for jc in range(2):
                for kt in range(N_KT):
                    mm(p, t, kt, jc)
                evict_jc(t, p, jc)

    nc.compile()
    return nc


def _get_nc():
    if "v4" not in _BUILT:
        _BUILT["v4"] = _build_nc()
    return _BUILT["v4"]


LAST_RUN = {}


def _install_axon_ntff_shim():
    """Provide the missing ``antenv.axon_hooks`` module so
    ``run_bass_kernel_spmd(trace=True)`` can capture NTFF profiles under
    axon.  The hook drives ``axon_{start,stop}_nrt_profile`` in
    libaxon_pjrt.so directly (same ABI trn_boot uses)."""
    import contextlib
    import ctypes
    import sys
    import types

    if "antenv.axon_hooks" in sys.modules:
        return
    so_path = "/opt/axon/libaxon_pjrt.so"
    lib = ctypes.CDLL(so_path)
    if not hasattr(lib, "axon_start_nrt_profile"):
        raise RuntimeError("libaxon_pjrt.so lacks axon_start_nrt_profile")
    lib.axon_start_nrt_profile.argtypes = [
        ctypes.POINTER(ctypes.c_int64),
        ctypes.c_size_t,
    ]
    lib.axon_start_nrt_profile.restype = ctypes.c_int64
    lib.axon_stop_nrt_profile.argtypes = [ctypes.c_char_p]
    lib.axon_stop_nrt_profile.restype = ctypes.c_int64

    @contextlib.contextmanager
    def _hook(output_dir, device_ids):
        import jax

        jax.devices()
        if device_ids:
            ids = (ctypes.c_int64 * len(device_ids))(*device_ids)
            rc = lib.axon_start_nrt_profile(ids, len(device_ids))
        else:
            rc = lib.axon_start_nrt_profile(None, 0)
        if rc != 0:
            raise RuntimeError(f"axon_start_nrt_profile rc={rc}")
        try:
            yield
        finally:
            n = lib.axon_stop_nrt_profile(str(output_dir).encode())
            print(f"ntff profile: {n} file(s) written to {output_dir}")

    mod = types.ModuleType("antenv.axon_hooks")
    mod.get_axon_ntff_profile_hook = lambda: _hook
    mod.set_axon_ntff_profile_hook = lambda h: None
    sys.modules["antenv.axon_hooks"] = mod
    import antenv

    antenv.axon_hooks = mod


def kernel(x, perm_logit, abcd, b, _trace=False):
    import ml_dtypes
    import concourse.bass_utils as bass_utils
    from concourse.bass_utils import run_bass_kernel_spmd

    if _trace:
        try:
            _install_axon_ntff_shim()
            bass_utils.upload_artifacts = lambda tmpdir: tmpdir
        except Exception as e:  # degrade to untraced run
            print("trace setup failed:", e)
            _trace = False

    x = np.ascontiguousarray(np.asarray(x, np.float32))
    M = _build_matrix(perm_logit, abcd)  # [k, j] f32

    # [k, j] -> [p, kt, j] with k = kt*128 + p
    m_in = np.ascontiguousarray(
        M.reshape(N_KT, 128, N).transpose(1, 0, 2).astype(ml_dtypes.bfloat16)
    )

    xb = x.astype(ml_dtypes.bfloat16)  # [B_FULL, N]
    # per-core shard -> [p, t, kt, b] with row = t*128+b, col = kt*128+p
    def x_layout(a, c):
        s = a[c * B_CORE : (c + 1) * B_CORE]
        return np.ascontiguousarray(
            s.reshape(N_BTILES, 128, N_KT, 128).transpose(3, 0, 2, 1)
        )

    bias_in = np.ascontiguousarray(
        np.broadcast_to(np.asarray(b, np.float32), (128, N))
    )

    nc = _get_nc()
    in_maps = [
        {
            "xb": x_layout(xb, c),
            "m": m_in,
            "bias": bias_in,
        }
        for c in range(N_CORES)
    ]
    res = run_bass_kernel_spmd(
        nc, in_maps, core_ids=list(range(N_CORES)), trace=_trace
    )
    LAST_RUN["results"] = res
    LAST_RUN["exec_time_ns"] = res.exec_time_ns
    out = np.concatenate(
        [np.asarray(r["out"]).astype(np.float32) for r in res.results], axis=0
    )
    return out


# revision 8
# speedup vs baseline: 1.1231x; 1.0028x over previous
"""Butterfly (nn_Butterfly) forward as a single dense matmul on 8 TRN2 cores.

The reference butterfly network is linear in x: forward(x) == x @ M + b where
M = forward(I_1024) with b=0.  M is built on the host from the ~16KB params.

v4 device kernel (from v3 trace analysis: 12us DMA-limited ramp with a cold
PE, 8.5us end-of-kernel semaphore teardown, f32 stores):
  - bf16 stores (4.2MB instead of 8.4MB per core), host upcasts to f32.
  - 8 junk warm-up matmuls issued at t=0 into a scratch psum bank so the
    HAM clock gate reaches 8/8 (2.4 GHz) before the first real matmul.
  - loads split across two HWDGE rings: M (per-kt chunks) + bulk x on the
    sync ring, first x tiles on the vector ring, bias on gpsimd; stores on
    the scalar ring.  First real matmul gates only on m[kt0] + x[t0:2].
  - explicit tile reuse (3 psum set pairs, 2 output sbuf buffers) instead
    of per-btile pool.tile() calls -- the end-of-kernel teardown barrier
    scales with the number of tile objects (~52 in v3).
  - kt-outer/jc-inner steady state: one x LDWEIGHTS serves both jc matmuls.
"""

import numpy as np

N = 1024
B_FULL = 16384
N_CORES = 8
B_CORE = B_FULL // N_CORES  # 2048
N_BTILES = B_CORE // 128  # 16
N_KT = 8  # k-tiles (K=128 each)
N_WARMUP_MM = 5


# ---------------------------------------------------------------------------
# Host side: collapse the butterfly network to a single matrix
# ---------------------------------------------------------------------------

def _abcd_offsets(n):
    offs = []
    off = 0
    m = n
    while m >= 2:
        offs.append((m, off))
        off += 2 * m
        m //= 2
    return offs, off


def _np_forward(x, perm_logit, abcd, b):
    """Float64 numpy port of reference._forward (op-for-op)."""
    x = np.asarray(x, np.float64)
    perm_logit = np.asarray(perm_logit, np.float64)
    abcd = np.asarray(abcd, np.float64)
    b = np.asarray(b, np.float64)
    n = x.shape[-1]
    Bn = x.shape[0]
    offs, _ = _abcd_offsets(n)
    h = np.stack([x, np.zeros_like(x)], axis=-1)
    perm_sizes = [m for (m, _) in offs if m >= 4]
    for d in range(perm_logit.shape[0]):
        p = 1.0 / (1.0 + np.exp(-perm_logit[d]))
        for m in reversed(perm_sizes):
            h = h.reshape(Bn, n // m, m, 2)
            eo = np.concatenate([h[:, :, 0::2], h[:, :, 1::2]], axis=2)
            h = (1 - p[0]) * h + p[0] * eo
            h1, h2 = h[:, :, : m // 2], h[:, :, m // 2 :]
            h1 = (1 - p[1]) * h1 + p[1] * h1[:, :, ::-1]
            h2 = (1 - p[2]) * h2 + p[2] * h2[:, :, ::-1]
            h = np.concatenate([h1, h2], axis=2).reshape(Bn, n, 2)
        for (m, off) in reversed(offs):
            ABCD = abcd[d, off : off + 2 * m].reshape(2, 2, m // 2, 2)
            hv = h.reshape(Bn, n // m, 2, m // 2, 2)
            xr, xi = hv[..., 0], hv[..., 1]
            Ar, Ai = ABCD[..., 0], ABCD[..., 1]
            yr = np.einsum("ijk,bnjk->bnik", Ar, xr) - np.einsum(
                "ijk,bnjk->bnik", Ai, xi
            )
            yi = np.einsum("ijk,bnjk->bnik", Ar, xi) + np.einsum(
                "ijk,bnjk->bnik", Ai, xr
            )
            h = np.stack([yr, yi], axis=-1).reshape(Bn, n, 2)
    return b + h[..., 0]


def _build_matrix(perm_logit, abcd):
    """M (f32, [k, j]) with forward(x) == x @ M + b."""
    I = np.eye(N, dtype=np.float64)
    M = _np_forward(I, perm_logit, abcd, np.zeros((N,), np.float64))
    return M.astype(np.float32)


# ---------------------------------------------------------------------------
# Device kernel
# ---------------------------------------------------------------------------

_BUILT = {}


def _build_nc():
    import concourse.bacc as bacc
    import concourse.mybir as mybir
    from concourse.tile import TileContext

    f32 = mybir.dt.float32
    bf16 = mybir.dt.bfloat16

    nc = bacc.Bacc(None, target_bir_lowering=False)

    xb_d = nc.dram_tensor("xb", [128, N_BTILES, N_KT, 128], bf16, kind="ExternalInput")
    m_d = nc.dram_tensor("m", [128, N_KT, N], bf16, kind="ExternalInput")
    b_d = nc.dram_tensor("bias", [128, N], f32, kind="ExternalInput")
    o_d = nc.dram_tensor("out", [B_CORE, N], bf16, kind="ExternalOutput")

    with TileContext(nc) as tc:
        with (
            tc.tile_pool(name="const", bufs=1) as const,
            tc.tile_pool(name="ps", bufs=1, space="PSUM") as ppool,
        ):
            m_sb = const.tile([128, N_KT, N], bf16)
            xb_sb = const.tile([128, N_BTILES, N_KT, 128], bf16)
            bias_sb = const.tile([128, N], f32)
            junk_sb = const.tile([128, 512], bf16)
            out_sb = [
                const.tile([128, N], bf16, name=f"osb{i}", tag=f"osb{i}")
                for i in range(2)
            ]

            scratch_ps = ppool.tile([128, 512], f32, name="scratch", tag="scratch")
            po = [
                [
                    ppool.tile([128, 512], f32, name=f"po{s}_{jc}", tag=f"po{s}_{jc}")
                    for jc in range(2)
                ]
                for s in range(3)
            ]

            # PE warm-up: 8 junk matmuls (~3.4us at the cold 1.2 GHz clock)
            # flip the HAM clock gate to 8/8 while the first DMAs land.
            nc.gpsimd.memset(junk_sb[:], 0.0)
            for _ in range(N_WARMUP_MM):
                nc.tensor.matmul(
                    scratch_ps[:], junk_sb[:, 0:128], junk_sb[:], start=True, stop=True
                )

            # Loads in consumption order on the sync HWDGE ring: m[kt0],
            # the two ramp x btiles, then the remaining M chunks.  Bulk x
            # and bias go on the scalar HWDGE ring (in front of the
            # stores), so both rings stream concurrently from t=0.
            nc.sync.dma_start(m_sb[:, 0], m_d[:, 0])
            nc.sync.dma_start(xb_sb[:, 0:1], xb_d[:, 0:1])
            nc.sync.dma_start(m_sb[:, 1], m_d[:, 1])
            nc.sync.dma_start(xb_sb[:, 1:2], xb_d[:, 1:2])
            for kt in range(2, N_KT):
                nc.sync.dma_start(m_sb[:, kt], m_d[:, kt])
            for t in range(2, N_BTILES):
                nc.sync.dma_start(xb_sb[:, t : t + 1], xb_d[:, t : t + 1])
            nc.scalar.dma_start(bias_sb[:], b_d[:])

            def mm(p, t, kt, jc):
                js = slice(jc * 512, (jc + 1) * 512)
                nc.tensor.matmul(
                    p[jc][:],
                    xb_sb[:, t, kt, :],
                    m_sb[:, kt, js],
                    start=(kt == 0),
                    stop=(kt == N_KT - 1),
                )

            def evict_jc(t, p, jc):
                # per-jc add + store: each half leaves as soon as its psum
                # group closes, shortening the end-of-kernel tail.
                osb = out_sb[t % 2]
                js = slice(jc * 512, (jc + 1) * 512)
                nc.vector.tensor_add(osb[:, js], p[jc][:], bias_sb[:, js])
                nc.scalar.dma_start(o_d[t * 128 : (t + 1) * 128, js], osb[:, js])

            def evict(t, p):
                for jc in range(2):
                    evict_jc(t, p, jc)

            # Ramp: btiles 0-1 kt-outer, consuming each M chunk as it lands.
            for kt in range(N_KT):
                for t in range(2):
                    for jc in range(2):
                        mm(po[t], t, kt, jc)
            for t in range(2):
                evict(t, po[t])

            # Steady state: kt-outer / jc-inner (one LDWEIGHTS per x tile),
            # psum sets round-robin t%3, output sbuf ping-pong t%2.
            for t in range(2, N_BTILES - 1):
                p = po[t % 3]
                for kt in range(N_KT):
                    for jc in range(2):
                        mm(p, t, kt, jc)
                evict(t, p)

            # Last btile jc-outer: jc0 closes 8 matmuls early, so its
            # evict + store overlap jc1's matmuls.
            t = N_BTILES - 1
            p = po[t % 3]
            for jc in range(2):
                for kt in range(N_KT):
                    mm(p, t, kt, jc)
                evict_jc(t, p, jc)

    nc.compile()
    return nc


def _get_nc():
    if "v4" not in _BUILT:
        _BUILT["v4"] = _build_nc()
    return _BUILT["v4"]


LAST_RUN = {}


def _install_axon_ntff_shim():
    """Provide the missing ``antenv.axon_hooks`` module so
    ``run_bass_kernel_spmd(trace=True)`` can capture NTFF profiles under
    axon.  The hook drives ``axon_{start,stop}_nrt_profile`` in
    libaxon_pjrt.so directly (same ABI trn_boot uses)."""
    import contextlib
    import ctypes
    import sys
    import types

    if "antenv.axon_hooks" in sys.modules:
        return
    so_path = "/opt/axon/libaxon_pjrt.so"
    lib = ctypes.CDLL(so_path)
    if not hasattr(lib, "axon_start_nrt_profile"):
        raise RuntimeError("libaxon_pjrt.so lacks axon_start_nrt_profile")
    lib.axon_start_nrt_profile.argtypes = [
        ctypes.POINTER(ctypes.c_int64),
        ctypes.c_size_t,
    ]
    lib.axon_start_nrt_profile.restype = ctypes.c_int64
    lib.axon_stop_nrt_profile.argtypes = [ctypes.c_char_p]
    lib.axon_stop_nrt_profile.restype = ctypes.c_int64

    @contextlib.contextmanager
    def _hook(output_dir, device_ids):
        import jax

        jax.devices()
        if device_ids:
            ids = (ctypes.c_int64 * len(device_ids))(*device_ids)
            rc = lib.axon_start_nrt_profile(ids, len(device_ids))
        else:
            rc = lib.axon_start_nrt_profile(None, 0)
        if rc != 0:
            raise RuntimeError(f"axon_start_nrt_profile rc={rc}")
        try:
            yield
        finally:
            n = lib.axon_stop_nrt_profile(str(output_dir).encode())
            print(f"ntff profile: {n} file(s) written to {output_dir}")

    mod = types.ModuleType("antenv.axon_hooks")
    mod.get_axon_ntff_profile_hook = lambda: _hook
    mod.set_axon_ntff_profile_hook = lambda h: None
    sys.modules["antenv.axon_hooks"] = mod
    import antenv

    antenv.axon_hooks = mod


def kernel(x, perm_logit, abcd, b, _trace=False):
    import ml_dtypes
    import concourse.bass_utils as bass_utils
    from concourse.bass_utils import run_bass_kernel_spmd

    if _trace:
        try:
            _install_axon_ntff_shim()
            bass_utils.upload_artifacts = lambda tmpdir: tmpdir
        except Exception as e:  # degrade to untraced run
            print("trace setup failed:", e)
            _trace = False

    x = np.ascontiguousarray(np.asarray(x, np.float32))
    M = _build_matrix(perm_logit, abcd)  # [k, j] f32

    # [k, j] -> [p, kt, j] with k = kt*128 + p
    m_in = np.ascontiguousarray(
        M.reshape(N_KT, 128, N).transpose(1, 0, 2).astype(ml_dtypes.bfloat16)
    )

    xb = x.astype(ml_dtypes.bfloat16)  # [B_FULL, N]
    # per-core shard -> [p, t, kt, b] with row = t*128+b, col = kt*128+p
    def x_layout(a, c):
        s = a[c * B_CORE : (c + 1) * B_CORE]
        return np.ascontiguousarray(
            s.reshape(N_BTILES, 128, N_KT, 128).transpose(3, 0, 2, 1)
        )

    bias_in = np.ascontiguousarray(
        np.broadcast_to(np.asarray(b, np.float32), (128, N))
    )

    nc = _get_nc()
    in_maps = [
        {
            "xb": x_layout(xb, c),
            "m": m_in,
            "bias": bias_in,
        }
        for c in range(N_CORES)
    ]
    res = run_bass_kernel_spmd(
        nc, in_maps, core_ids=list(range(N_CORES)), trace=_trace
    )
    LAST_RUN["results"] = res
    LAST_RUN["exec_time_ns"] = res.exec_time_ns
    out = np.concatenate(
        [np.asarray(r["out"]).astype(np.float32) for r in res.results], axis=0
    )
    return out
